# revision 47
# baseline (speedup 1.0000x reference)
"""Trainium2 Bass kernel for nn_AbstractAtt (MLB-style 2-glimpse attention).

Strategy (8 NeuronCores, one SPMD NEFF):
- Phase 1 (attention) data-parallel over batch: 8 batches/core.
  xv = tanh(conv1x1(input_v)) computed transposed (DA on partitions) with
  bf16 matmuls (fp32 accumulate); xatt = tanh(xv * xq); wgt = xatt @ Watt.T
  (+batt) in fp32r; softmax over spatial; v_att = att-weighted sum of
  input_v in fp32r. xq is tensor-parallel over DA (150 rows/core) + AllToAll.
- Per-window AllGather of v_att (4 small gathers overlap compute).
- Phase 2 tensor-parallel over the 2400-dim fusion axis (300 cols/core),
  batches processed in gathered (window, core, b) order; host un-permutes.
  v_fus/q_fus slices, z = v_fus*q_fus, partial logits z @ Wc.T slice.
  Host sums the 8 partial logit tensors and adds bc.
"""
import numpy as np
import ml_dtypes

import concourse.bacc as bacc
import concourse.mybir as mybir
import concourse.tile as tile
from concourse.tile import add_dep_helper
from concourse.masks import make_identity
from concourse.bass_utils import run_bass_kernel_spmd

F32 = mybir.dt.float32
F32R = mybir.dt.float32r
BF16 = mybir.dt.bfloat16
AF = mybir.ActivationFunctionType
AX = mybir.AxisListType

NCORES = 8
B, DV, WH = 64, 2048, 196
DQ, DA, G, DH, NCLS = 2400, 1200, 2, 1200, 3000
BL = B // NCORES          # 8 local batches
NW = 4                    # windows per core
WB = BL // NW             # 2 batches per window
RW = WB * WH              # 392 rows per window
DVT = DV // 128           # 16 k-tiles over channels
DAT = 10                  # DA m-tiles: 9*128 + 48
DA_LAST = DA - 9 * 128    # 48
DQP = 2432                # DQ padded to 19*128
DQT = DQP // 128          # 19
DSL = DA // NCORES        # 150 (xq tensor-parallel slice)
GSL = (G * DH) // NCORES  # 300 (phase-2 slice)
GVT = 2 * DVT             # 32 k-tiles over (g, d) for v_fus
NCH = 6                   # classifier N chunks
NCW = NCLS // NCH         # 500

# phase-2 batch order: j = 16w + 2c + bw  <->  b_global = 8c + 2w + bw
PERM = np.zeros(B, np.int64)
for _j in range(B):
    _w, _r = divmod(_j, 2 * NCORES)
    _c, _bw = divmod(_r, WB)
    PERM[_j] = BL * _c + WB * _w + _bw


def _mt_rows(mt):
    return 128 if mt < DAT - 1 else DA_LAST


def build():
    nc = bacc.Bacc(None, num_devices=NCORES)

    # ---- per-core external inputs ----
    V = nc.declare_dram_parameter("v", [BL, DV, WH], BF16, isOutput=False)
    VnT = nc.declare_dram_parameter("vnt", [BL, WH, DV], BF16, isOutput=False)
    WvT = nc.declare_dram_parameter("wvt", [DV, DA], BF16, isOutput=False)
    BVA = nc.declare_dram_parameter("bva", [1280], F32, isOutput=False)
    WqaT = nc.declare_dram_parameter("wqat", [DQP, DA], BF16, isOutput=False)
    BQA = nc.declare_dram_parameter("bqa", [1280], F32, isOutput=False)
    QTL = nc.declare_dram_parameter("qtl", [DQP, BL], BF16, isOutput=False)
    QT2 = nc.declare_dram_parameter("qt2", [DQP, B], BF16, isOutput=False)
    WattT = nc.declare_dram_parameter("watt", [1280, G], F32R, isOutput=False)
    BATT = nc.declare_dram_parameter("batt", [G, 1], F32, isOutput=False)
    WvfT = nc.declare_dram_parameter("wvft", [2 * DV, GSL], BF16, isOutput=False)
    BVF = nc.declare_dram_parameter("bvf", [384], F32, isOutput=False)
    WqfT = nc.declare_dram_parameter("wqft", [DQP, GSL], BF16, isOutput=False)
    BQF = nc.declare_dram_parameter("bqf", [384], F32, isOutput=False)
    WcT = nc.declare_dram_parameter("wct", [384, NCLS], BF16, isOutput=False)

    # ---- per-core external outputs (x_part rows in PERM order) ----
    WGT_O = nc.declare_dram_parameter("wgt_out", [BL, G, WH], F32, isOutput=True)
    X_O = nc.declare_dram_parameter("x_part", [B, NCLS], F32, isOutput=True)

    # ---- internal DRAM for collectives ----
    vatt_loc = nc.dram_tensor("vatt_loc", [BL, G, DV], BF16)
    # gathered per window: [w][(core, bw)][g][d]
    vatt_all = nc.dram_tensor(
        "vatt_all", [NW, 2 * NCORES, G, DV], BF16, addr_space="Shared"
    )

    with tile.TileContext(nc) as tc:
        with (
            tc.tile_pool(name="const", bufs=1) as const,
            tc.tile_pool(name="ph2w", bufs=1) as ph2w,
            tc.tile_pool(name="pstr", bufs=1, space="PSUM") as pstr,
        ):
            ident = const.tile([128, 128], F32)
            make_identity(nc, ident)
            # persistent small tiles
            xqT_sb = const.tile([128, DAT, BL], F32)      # xq for local batches
            WattT_sb = const.tile([128, DAT, G], F32R)
            bva_sb = const.tile([128, DAT], F32)
            batt_sb = const.tile([G, 1], F32)
            qf = const.tile([128, 3, B], F32)             # q_fus (PERM order)
            # phase-2 prefetch tiles (DMA'd on the scalar queue during phase A)
            WqfT_sb = ph2w.tile([128, DQT, GSL], BF16)
            qT2_sb = ph2w.tile([128, DQT, B], BF16)
            bqf_sb = ph2w.tile([128, 3], F32)
            WvfT_sb = ph2w.tile([128, GVT, GSL], BF16)
            WcT_sb = ph2w.tile([128, 3, NCLS], BF16)
            bvf_sb = ph2w.tile([128, 3], F32)
            vattT_sb = ph2w.tile([128, GVT, B], BF16)

            nc.sync.dma_start(
                WattT_sb[:], WattT.rearrange("(t p) g -> p t g", p=128)
            )
            nc.sync.dma_start(bva_sb[:], BVA.rearrange("(t p) -> p t", p=128))
            nc.sync.dma_start(batt_sb[:], BATT[:])

            # ===== phase A: xq (tensor-parallel over DA) + early q_fus ======
            with (
                tc.tile_pool(name="xqp", bufs=1) as xqp,
                tc.tile_pool(name="psxq", bufs=2, space="PSUM") as psxq,
            ):
                WqaT_sb = xqp.tile([128, DQT, DA], BF16)
                qTl_sb = xqp.tile([128, DQT, BL], BF16)
                nc.scalar.dma_start(
                    qTl_sb[:], QTL.rearrange("(t p) b -> p t b", p=128)
                )
                bqa_sb = xqp.tile([128, DAT], F32)
                nc.scalar.dma_start(bqa_sb[:], BQA.rearrange("(c p) -> p c", p=128))
                # per-kt WqaT loads so the first xq matmul starts after ~0.3MB
                for kt in range(DQT):
                    nc.scalar.dma_start(
                        WqaT_sb[:, kt, :], WqaT[kt * 128 : (kt + 1) * 128, :]
                    )
                for mt in range(DAT):
                    rows = _mt_rows(mt)
                    ps = psxq.tile([128, BL], F32, tag="psxq")
                    for kt in range(DQT):
                        nc.tensor.matmul(
                            ps[:rows, :],
                            WqaT_sb[:, kt, mt * 128 : mt * 128 + rows],
                            qTl_sb[:, kt, :],
                            start=(kt == 0),
                            stop=(kt == DQT - 1),
                        )
                    nc.scalar.activation(
                        xqT_sb[:rows, mt, :], ps[:rows, :], AF.Tanh,
                        bias=bqa_sb[:rows, mt : mt + 1],
                    )
                # prefetch phase-2 weights on the scalar queue (behind the
                # xq inputs, ahead of nothing critical)
                nc.scalar.dma_start(
                    WqfT_sb[:], WqfT.rearrange("(t p) m -> p t m", p=128)
                )
                nc.scalar.dma_start(
                    qT2_sb[:], QT2.rearrange("(t p) b -> p t b", p=128)
                )
                nc.scalar.dma_start(bqf_sb[:], BQF.rearrange("(c p) -> p c", p=128))
                nc.scalar.dma_start(
                    WvfT_sb[:], WvfT.rearrange("(t p) m -> p t m", p=128)
                )
                nc.scalar.dma_start(
                    WcT_sb[:], WcT.rearrange("(t p) n -> p t n", p=128)
                )
                nc.scalar.dma_start(bvf_sb[:], BVF.rearrange("(c p) -> p c", p=128))

            # ===== phase B: windows (xv -> xatt -> wgt -> att -> v_att) =====
            ccv = []
            with tc.tile_pool(name="wvtp", bufs=1) as wvtp:
                WvT_sb = wvtp.tile([128, DVT, DA], BF16)
                for kt in range(DVT):
                    nc.sync.dma_start(
                        WvT_sb[:, kt, :], WvT[kt * 128 : (kt + 1) * 128, :]
                    )
                with (
                    tc.tile_pool(name="vwin", bufs=2) as vwp,
                    tc.tile_pool(name="vntp", bufs=1) as vntp,
                    tc.tile_pool(name="xatp", bufs=1) as xap,
                    tc.tile_pool(name="tmpp", bufs=3) as tmpp,
                    tc.tile_pool(name="psxv", bufs=3, space="PSUM") as psxv,
                    tc.tile_pool(name="pswg", bufs=1, space="PSUM") as pswg,
                    tc.tile_pool(name="psva", bufs=1, space="PSUM") as psva,
                ):
                    for w in range(NW):
                        V_w = vwp.tile([128, DVT, WB, WH], BF16, tag="vw")
                        for bw in range(WB):
                            nc.sync.dma_start(
                                V_w[:, :, bw, :],
                                V[2 * w + bw].rearrange("(t p) n -> p t n", p=128),
                            )
                        VnT0 = vntp.tile([128, WB, DV], BF16, tag="vnt0")
                        VnT1 = vntp.tile([68, WB, DV], BF16, tag="vnt1")
                        nc.sync.dma_start(
                            VnT0[:],
                            VnT[2 * w : 2 * w + 2, 0:128, :].rearrange(
                                "b p d -> p b d"
                            ),
                        )
                        nc.sync.dma_start(
                            VnT1[:],
                            VnT[2 * w : 2 * w + 2, 128:WH, :].rearrange(
                                "b p d -> p b d"
                            ),
                        )
                        xatt_w = xap.tile([128, DAT, RW], F32R, tag="xatt")
                        for mt in range(DAT):
                            rows = _mt_rows(mt)
                            ps = psxv.tile([128, RW], F32, tag="psxv")
                            for kt in range(DVT):
                                nc.tensor.matmul(
                                    ps[:rows, :],
                                    WvT_sb[:, kt, mt * 128 : mt * 128 + rows],
                                    V_w[:, kt].rearrange("p b n -> p (b n)"),
                                    start=(kt == 0),
                                    stop=(kt == DVT - 1),
                                )
                            t1 = tmpp.tile([128, WB, WH], F32, tag="t1")
                            nc.scalar.activation(
                                t1[:rows].rearrange("p b n -> p (b n)"),
                                ps[:rows, :],
                                AF.Tanh, bias=bva_sb[:rows, mt : mt + 1],
                            )
                            nc.vector.tensor_tensor(
                                t1[:rows],
                                t1[:rows],
                                xqT_sb[:rows, mt, 2 * w : 2 * w + 2, None]
                                .to_broadcast([rows, WB, WH]),
                                mybir.AluOpType.mult,
                            )
                            nc.scalar.activation(
                                xatt_w[:rows, mt, :],
                                t1[:rows].rearrange("p b n -> p (b n)"),
                                AF.Tanh,
                            )
                        if w == 0:
                            # q_fus fills the PE gap while window 0's
                            # eviction chain (ACT/DVE) catches up
                            for ch, rows in enumerate((128, 128, GSL - 256)):
                                psq = pswg.tile([128, B], F32, tag="psqf")
                                for kt in range(DQT):
                                    nc.tensor.matmul(
                                        psq[:rows, :],
                                        WqfT_sb[:, kt, ch * 128 : ch * 128 + rows],
                                        qT2_sb[:, kt, :],
                                        start=(kt == 0), stop=(kt == DQT - 1),
                                    )
                                nc.scalar.activation(
                                    qf[:rows, ch, :], psq[:rows, :], AF.Tanh,
                                    bias=bqf_sb[:rows, ch : ch + 1],
                                )
                        # wgt = xatt @ Watt.T  -> psum [G, RW]
                        psw = pswg.tile([G, RW], F32, tag="pswg")
                        for mt in range(DAT):
                            rows = _mt_rows(mt)
                            nc.tensor.matmul(
                                psw[:, :],
                                WattT_sb[:rows, mt, :],
                                xatt_w[:rows, mt, :],
                                start=(mt == 0),
                                stop=(mt == DAT - 1),
                            )
                        # biased logits [g, b, n]; DMA out per window
                        wgt_w = tmpp.tile([G, WB, WH], F32, tag="wgtw", bufs=2)
                        nc.scalar.activation(
                            wgt_w[:],
                            psw[:].rearrange("g (b n) -> g b n", b=WB),
                            AF.Identity, bias=batt_sb[:],
                        )
                        nc.sync.dma_start(
                            WGT_O[2 * w : 2 * w + 2].rearrange("b g n -> g b n"),
                            wgt_w[:],
                        )
                        # softmax over n in [g, b, n] layout (all offsets 0)
                        mx = tmpp.tile([G, WB, 1], F32, tag="mx", bufs=2)
                        sm = tmpp.tile([G, WB, 1], F32, tag="sm", bufs=2)
                        rs = tmpp.tile([G, WB, 1], F32, tag="rs", bufs=2)
                        att_w = tmpp.tile([G, WB, WH], F32, tag="attw", bufs=2)
                        nc.vector.reduce_max(mx[:], wgt_w[:], axis=AX.X, negate=True)
                        nc.vector.tensor_tensor(
                            att_w[:], wgt_w[:], mx[:].to_broadcast([G, WB, WH]),
                            mybir.AluOpType.add,
                        )
                        nc.scalar.activation(att_w[:], att_w[:], AF.Exp)
                        nc.vector.reduce_sum(sm[:], att_w[:], axis=AX.X)
                        nc.vector.reciprocal(rs[:], sm[:])
                        nc.vector.tensor_tensor(
                            att_w[:], att_w[:], rs[:].to_broadcast([G, WB, WH]),
                            mybir.AluOpType.mult,
                        )
                        # transpose att -> 4 block-column lhsT tiles [n, (bw, g)]
                        # (zero cols for the other batch; keeps PSUM dst at
                        # base partition 0, which the fp32r matmul requires)
                        attTs = []
                        for bw in range(WB):
                            a0 = tmpp.tile([128, 2 * WB], BF16,
                                           tag=f"attT0{bw}", bufs=2)
                            a1 = tmpp.tile([68, 2 * WB], BF16,
                                           tag=f"attT1{bw}", bufs=2)
                            nc.vector.memset(a0[:], 0.0)
                            nc.vector.memset(a1[:], 0.0)
                            pt0 = pstr.tile([128, G], F32, tag="pstr")
                            nc.tensor.transpose(
                                pt0[:], att_w[:, bw, 0:128], ident[0:G, 0:G]
                            )
                            nc.vector.tensor_copy(
                                a0[:, 2 * bw : 2 * bw + 2], pt0[:]
                            )
                            pt1 = pstr.tile([128, G], F32, tag="pstr")
                            nc.tensor.transpose(
                                pt1[0:68, :], att_w[:, bw, 128:WH], ident[0:G, 0:G]
                            )
                            nc.vector.tensor_copy(
                                a1[:, 2 * bw : 2 * bw + 2], pt1[0:68, :]
                            )
                            attTs.append((a0, a1))
                        # v_att: accumulate all (bw, n-chunk) into psum [4, 512]
                        vatt_w = tmpp.tile([2 * WB, DV], BF16, tag="vattw", bufs=1)
                        for ch in range(DV // 512):
                            pv = psva.tile([2 * WB, 512], F32, tag="psva")
                            for bw in range(WB):
                                a0, a1 = attTs[bw]
                                nc.tensor.matmul(
                                    pv[:, :], a0[:, :],
                                    VnT0[:, bw, ch * 512 : (ch + 1) * 512],
                                    start=(bw == 0), stop=False,
                                )
                                nc.tensor.matmul(
                                    pv[:, :], a1[:, :],
                                    VnT1[:, bw, ch * 512 : (ch + 1) * 512],
                                    start=False, stop=(bw == WB - 1),
                                )
                            nc.vector.tensor_copy(
                                vatt_w[:, ch * 512 : (ch + 1) * 512], pv[:]
                            )
                        nc.sync.dma_start(
                            vatt_loc[2 * w : 2 * w + 2].rearrange(
                                "b g d -> (b g) d"
                            ),
                            vatt_w[:],
                        )
                        # per-window AllGather (overlaps later windows)
                        ccw = nc.gpsimd.collective_compute(
                            "AllGather", mybir.AluOpType.bypass,
                            replica_groups=[list(range(NCORES))],
                            ins=[vatt_loc[2 * w : 2 * w + 2]],
                            outs=[vatt_all[w]],
                        )
                        ccv.append(ccw)

            # ================= phase C: fusion + classifier =================
            with (
                tc.tile_pool(name="ph2", bufs=1) as ph2,
                tc.tile_pool(name="ph2t", bufs=2) as ph2t,
                tc.tile_pool(name="psf", bufs=2, space="PSUM") as psf,
                tc.tile_pool(name="psx", bufs=2, space="PSUM") as psx,
            ):
                # v_att (both glimpses), batches in PERM order
                vatt_bg = ph2.tile([64, GVT, 128], BF16)
                d3 = nc.sync.dma_start(
                    vatt_bg[:],
                    vatt_all.rearrange("w c g (t p) -> (w c) (g t) p", p=128),
                )
                for ccw in ccv:
                    add_dep_helper(d3.ins, ccw.ins, reason="vatt allgather -> read")
                identb = ph2.tile([64, 64], BF16)
                nc.vector.tensor_copy(identb[:], ident[0:64, 0:64])
                for t in range(GVT):
                    ptv = pstr.tile([128, 64], BF16, tag="pstr")
                    nc.tensor.transpose(ptv[:], vatt_bg[:, t, :], identb[:])
                    nc.vector.tensor_copy(vattT_sb[:, t, :], ptv[:])

                # v_fus / z
                zT = ph2.tile([128, 3, B], BF16)
                nc.vector.memset(zT[:].bitcast(F32), 0.0)
                vf = ph2t.tile([128, 3, B], F32, tag="vf")
                for ch, rows in enumerate((128, 128, GSL - 256)):
                    psv = psf.tile([128, B], F32, tag="psv")
                    for kt in range(GVT):
                        nc.tensor.matmul(
                            psv[:rows, :],
                            WvfT_sb[:, kt, ch * 128 : ch * 128 + rows],
                            vattT_sb[:, kt, :],
                            start=(kt == 0), stop=(kt == GVT - 1),
                        )
                    nc.scalar.activation(
                        vf[:rows, ch, :], psv[:rows, :], AF.Tanh,
                        bias=bvf_sb[:rows, ch : ch + 1],
                    )
                    nc.vector.tensor_tensor(
                        zT[:rows, ch, :], vf[:rows, ch, :], qf[:rows, ch, :],
                        mybir.AluOpType.mult,
                    )
                # classifier partials: x [B, NCLS] (rows in PERM order)
                x_sb = ph2.tile([64, NCLS], F32)
                for ch in range(NCH):
                    px = psx.tile([64, NCW], F32, tag="px")
                    for kt in range(3):
                        nc.tensor.matmul(
                            px[:, :],
                            zT[:, kt, :],
                            WcT_sb[:, kt, ch * NCW : (ch + 1) * NCW],
                            start=(kt == 0), stop=(kt == 2),
                        )
                    nc.vector.tensor_copy(
                        x_sb[:, ch * NCW : (ch + 1) * NCW], px[:]
                    )
                nc.sync.dma_start(X_O[:], x_sb[:])

    nc.compile()
    return nc


_NC_CACHE = None


def _get_nc():
    global _NC_CACHE
    if _NC_CACHE is None:
        _NC_CACHE = build()
    return _NC_CACHE


def _prep_inputs(input_q, input_v, Wv_att, bv_att, Wq_att, bq_att, Watt, batt,
                 Wv_fus, bv_fus, Wq_fus, bq_fus, Wc, bc):
    """Host-side prep: transposes, pads, per-core slices."""
    f = np.float32
    bf = ml_dtypes.bfloat16
    input_q = np.ascontiguousarray(input_q, dtype=f)
    input_v = np.ascontiguousarray(input_v, dtype=f)

    v_flat = input_v.reshape(B, DV, WH)
    WvT_h = np.ascontiguousarray(Wv_att.T, dtype=bf)             # [2048, 1200]
    bva_h = np.zeros(1280, f)
    bva_h[:DA] = bv_att
    qT_h = np.zeros((DQP, B), np.float32)
    qT_h[:DQ] = input_q.T
    qT2_h = np.ascontiguousarray(qT_h[:, PERM]).astype(bf)
    WqaT_h = np.zeros((DQP, DA), bf)
    WqaT_h[:DQ] = Wq_att.T
    bqa_h = np.zeros(1280, f)
    bqa_h[:DA] = bq_att
    WattT_h = np.zeros((1280, G), f)
    WattT_h[:DA] = Watt.T
    batt_h = np.ascontiguousarray(batt.reshape(G, 1), dtype=f)

    in_maps = []
    for c in range(NCORES):
        bs = c * BL
        vloc = np.ascontiguousarray(v_flat[bs : bs + BL], dtype=bf)
        vnt = np.ascontiguousarray(
            v_flat[bs : bs + BL].transpose(0, 2, 1), dtype=bf)

        qTl_h = np.zeros((DQP, BL), bf)
        qTl_h[:DQ] = input_q[bs : bs + BL].T

        s0 = c * GSL
        g = s0 // DH
        h0 = s0 - g * DH
        WvfT_h = np.zeros((2 * DV, GSL), bf)
        WvfT_h[g * DV : (g + 1) * DV] = Wv_fus[g, h0 : h0 + GSL].T
        bvf_h = np.zeros(384, f)
        bvf_h[:GSL] = bv_fus[g, h0 : h0 + GSL]
        WqfT_h = np.zeros((DQP, GSL), bf)
        WqfT_h[:DQ] = Wq_fus[s0 : s0 + GSL].T
        bqf_h = np.zeros(384, f)
        bqf_h[:GSL] = bq_fus[s0 : s0 + GSL]
        WcT_h = np.zeros((384, NCLS), bf)
        WcT_h[:GSL] = Wc[:, s0 : s0 + GSL].T

        in_maps.append({
            "v": vloc, "vnt": vnt, "wvt": WvT_h, "bva": bva_h,
            "wqat": WqaT_h, "bqa": bqa_h, "qtl": qTl_h, "qt2": qT2_h,
            "watt": WattT_h, "batt": batt_h,
            "wvft": WvfT_h, "bvf": bvf_h, "wqft": WqfT_h, "bqf": bqf_h,
            "wct": WcT_h,
        })
    return in_maps


def kernel(**inputs):
    nc = _get_nc()
    in_maps = _prep_inputs(**{k: np.asarray(v) for k, v in inputs.items()})
    res = run_bass_kernel_spmd(nc, in_maps, core_ids=list(range(NCORES)))
    x = np.zeros((B, NCLS), np.float32)
    wgt = np.zeros((B, G, WH), np.float32)
    for c in range(NCORES):
        r = res.results[c]
        x += r["x_part"]
        wgt[c * BL : (c + 1) * BL] = r["wgt_out"]
    # un-permute: x_part rows are in PERM order (row j -> batch PERM[j])
    xo = np.zeros_like(x)
    xo[PERM] = x
    xo += np.asarray(inputs["bc"], np.float32)[None, :]
    return xo, wgt


# revision 48
# speedup vs baseline: 1.0428x; 1.0428x over previous
"""Trainium2 Bass kernel for nn_AbstractAtt (MLB-style 2-glimpse attention).

Strategy (8 NeuronCores, one SPMD NEFF):
- Phase 1 (attention) data-parallel over batch: 8 batches/core.
  xv = tanh(conv1x1(input_v)) computed transposed (DA on partitions) with
  bf16 matmuls (fp32 accumulate); xatt = tanh(xv * xq); wgt = xatt @ Watt.T
  (+batt) in fp32r; softmax over spatial; v_att = att-weighted sum of
  input_v in fp32r. xq is tensor-parallel over DA (150 rows/core) + AllToAll.
- Per-window AllGather of v_att (4 small gathers overlap compute).
- Phase 2 tensor-parallel over the 2400-dim fusion axis (300 cols/core),
  batches processed in gathered (window, core, b) order; host un-permutes.
  v_fus/q_fus slices, z = v_fus*q_fus, partial logits z @ Wc.T slice.
  Host sums the 8 partial logit tensors and adds bc.
"""
import numpy as np
import ml_dtypes

import concourse.bacc as bacc
import concourse.mybir as mybir
import concourse.tile as tile
from concourse.tile import add_dep_helper
from concourse.masks import make_identity
from concourse.bass_utils import run_bass_kernel_spmd

F32 = mybir.dt.float32
F32R = mybir.dt.float32r
BF16 = mybir.dt.bfloat16
AF = mybir.ActivationFunctionType
AX = mybir.AxisListType

NCORES = 8
B, DV, WH = 64, 2048, 196
DQ, DA, G, DH, NCLS = 2400, 1200, 2, 1200, 3000
BL = B // NCORES          # 8 local batches
NW = 4                    # windows per core
WB = BL // NW             # 2 batches per window
RW = WB * WH              # 392 rows per window
DVT = DV // 128           # 16 k-tiles over channels
DAT = 10                  # DA m-tiles: 9*128 + 48
DA_LAST = DA - 9 * 128    # 48
DQP = 2432                # DQ padded to 19*128
DQT = DQP // 128          # 19
DSL = DA // NCORES        # 150 (xq tensor-parallel slice)
GSL = (G * DH) // NCORES  # 300 (phase-2 slice)
GVT = 2 * DVT             # 32 k-tiles over (g, d) for v_fus
NCH = 6                   # classifier N chunks
NCW = NCLS // NCH         # 500

# phase-2 batch order: j = 16w + 2c + bw  <->  b_global = 8c + 2w + bw
PERM = np.zeros(B, np.int64)
for _j in range(B):
    _w, _r = divmod(_j, 2 * NCORES)
    _c, _bw = divmod(_r, WB)
    PERM[_j] = BL * _c + WB * _w + _bw


def _mt_rows(mt):
    return 128 if mt < DAT - 1 else DA_LAST


def build():
    nc = bacc.Bacc(None, num_devices=NCORES)

    # ---- per-core external inputs ----
    V = nc.declare_dram_parameter("v", [BL, DV, WH], BF16, isOutput=False)
    VnT = nc.declare_dram_parameter("vnt", [BL, WH, DV], BF16, isOutput=False)
    WvT = nc.declare_dram_parameter("wvt", [DV, DA], BF16, isOutput=False)
    BVA = nc.declare_dram_parameter("bva", [1280], F32, isOutput=False)
    WqaT = nc.declare_dram_parameter("wqat", [DQP, DA], BF16, isOutput=False)
    BQA = nc.declare_dram_parameter("bqa", [1280], F32, isOutput=False)
    QTL = nc.declare_dram_parameter("qtl", [DQP, BL], BF16, isOutput=False)
    QT2 = nc.declare_dram_parameter("qt2", [DQP, B], BF16, isOutput=False)
    WattT = nc.declare_dram_parameter("watt", [1280, G], F32R, isOutput=False)
    BATT = nc.declare_dram_parameter("batt", [G, 1], F32, isOutput=False)
    WvfT = nc.declare_dram_parameter("wvft", [2 * DV, GSL], BF16, isOutput=False)
    BVF = nc.declare_dram_parameter("bvf", [384], F32, isOutput=False)
    WqfT = nc.declare_dram_parameter("wqft", [DQP, GSL], BF16, isOutput=False)
    BQF = nc.declare_dram_parameter("bqf", [384], F32, isOutput=False)
    WcT = nc.declare_dram_parameter("wct", [384, NCLS], BF16, isOutput=False)

    # ---- per-core external outputs (x_part rows in PERM order) ----
    WGT_O = nc.declare_dram_parameter("wgt_out", [BL, G, WH], F32, isOutput=True)
    X_O = nc.declare_dram_parameter("x_part", [B, NCLS], F32, isOutput=True)

    # ---- internal DRAM for collectives ----
    vatt_loc = nc.dram_tensor("vatt_loc", [BL, G, DV], BF16)
    # gathered per window: [w][(core, bw)][g][d]
    vatt_all = nc.dram_tensor(
        "vatt_all", [NW, 2 * NCORES, G, DV], BF16, addr_space="Shared"
    )

    with tile.TileContext(nc) as tc:
        with (
            tc.tile_pool(name="const", bufs=1) as const,
            tc.tile_pool(name="ph2w", bufs=1) as ph2w,
            tc.tile_pool(name="pstr", bufs=1, space="PSUM") as pstr,
        ):
            ident = const.tile([128, 128], F32)
            make_identity(nc, ident)
            # persistent small tiles
            xqT_sb = const.tile([128, DAT, BL], F32)      # xq for local batches
            WattT_sb = const.tile([128, DAT, G], F32R)
            bva_sb = const.tile([128, DAT], F32)
            batt_sb = const.tile([G, 1], F32)
            qf = const.tile([128, 3, B], F32)             # q_fus (PERM order)
            # phase-2 prefetch tiles (DMA'd on the scalar queue during phase A)
            WqfT_sb = ph2w.tile([128, DQT, GSL], BF16)
            qT2_sb = ph2w.tile([128, DQT, B], BF16)
            bqf_sb = ph2w.tile([128, 3], F32)
            WvfT_sb = ph2w.tile([128, GVT, GSL], BF16)
            WcT_sb = ph2w.tile([128, 3, NCLS], BF16)
            bvf_sb = ph2w.tile([128, 3], F32)
            vattT_sb = ph2w.tile([128, GVT, B], BF16)

            nc.sync.dma_start(
                WattT_sb[:], WattT.rearrange("(t p) g -> p t g", p=128)
            )
            nc.sync.dma_start(bva_sb[:], BVA.rearrange("(t p) -> p t", p=128))
            nc.sync.dma_start(batt_sb[:], BATT[:])

            # ===== phase A: xq (tensor-parallel over DA) + early q_fus ======
            with (
                tc.tile_pool(name="xqp", bufs=1) as xqp,
                tc.tile_pool(name="psxq", bufs=2, space="PSUM") as psxq,
            ):
                WqaT_sb = xqp.tile([128, DQT, DA], BF16)
                qTl_sb = xqp.tile([128, DQT, BL], BF16)
                nc.scalar.dma_start(
                    qTl_sb[:], QTL.rearrange("(t p) b -> p t b", p=128)
                )
                bqa_sb = xqp.tile([128, DAT], F32)
                nc.scalar.dma_start(bqa_sb[:], BQA.rearrange("(c p) -> p c", p=128))
                # per-kt WqaT loads so the first xq matmul starts after ~0.3MB
                for kt in range(DQT):
                    nc.scalar.dma_start(
                        WqaT_sb[:, kt, :], WqaT[kt * 128 : (kt + 1) * 128, :]
                    )
                for mt in range(DAT):
                    rows = _mt_rows(mt)
                    ps = psxq.tile([128, BL], F32, tag="psxq")
                    for kt in range(DQT):
                        nc.tensor.matmul(
                            ps[:rows, :],
                            WqaT_sb[:, kt, mt * 128 : mt * 128 + rows],
                            qTl_sb[:, kt, :],
                            start=(kt == 0),
                            stop=(kt == DQT - 1),
                        )
                    nc.scalar.activation(
                        xqT_sb[:rows, mt, :], ps[:rows, :], AF.Tanh,
                        bias=bqa_sb[:rows, mt : mt + 1],
                    )
                # prefetch phase-2 weights on the scalar queue (behind the
                # xq inputs, ahead of nothing critical)
                nc.scalar.dma_start(
                    WqfT_sb[:], WqfT.rearrange("(t p) m -> p t m", p=128)
                )
                nc.scalar.dma_start(
                    qT2_sb[:], QT2.rearrange("(t p) b -> p t b", p=128)
                )
                nc.scalar.dma_start(bqf_sb[:], BQF.rearrange("(c p) -> p c", p=128))
                nc.scalar.dma_start(
                    WvfT_sb[:], WvfT.rearrange("(t p) m -> p t m", p=128)
                )
                nc.scalar.dma_start(
                    WcT_sb[:], WcT.rearrange("(t p) n -> p t n", p=128)
                )
                nc.scalar.dma_start(bvf_sb[:], BVF.rearrange("(c p) -> p c", p=128))
                # early q_fus (PERM batch order via QT2); weights prefetched
                # on the scalar DMA queue so the sync queue streams V/WvT.
                for ch, rows in enumerate((128, 128, GSL - 256)):
                    psq = psxq.tile([128, B], F32, tag="psxq")
                    for kt in range(DQT):
                        nc.tensor.matmul(
                            psq[:rows, :],
                            WqfT_sb[:, kt, ch * 128 : ch * 128 + rows],
                            qT2_sb[:, kt, :],
                            start=(kt == 0), stop=(kt == DQT - 1),
                        )
                    nc.scalar.activation(
                        qf[:rows, ch, :], psq[:rows, :], AF.Tanh,
                        bias=bqf_sb[:rows, ch : ch + 1],
                    )

            # ===== phase B: windows (xv -> xatt -> wgt -> att -> v_att) =====
            ccv = []
            with tc.tile_pool(name="wvtp", bufs=1) as wvtp:
                WvT_sb = wvtp.tile([128, DVT, DA], BF16)
                for kt in range(DVT):
                    nc.sync.dma_start(
                        WvT_sb[:, kt, :], WvT[kt * 128 : (kt + 1) * 128, :]
                    )
                with (
                    tc.tile_pool(name="vwin", bufs=2) as vwp,
                    tc.tile_pool(name="vntp", bufs=1) as vntp,
                    tc.tile_pool(name="xatp", bufs=1) as xap,
                    tc.tile_pool(name="tmpp", bufs=3) as tmpp,
                    tc.tile_pool(name="psxv", bufs=3, space="PSUM") as psxv,
                    tc.tile_pool(name="pswg", bufs=1, space="PSUM") as pswg,
                    tc.tile_pool(name="psva", bufs=1, space="PSUM") as psva,
                ):
                    for w in range(NW):
                        V_w = vwp.tile([128, DVT, WB, WH], BF16, tag="vw")
                        for bw in range(WB):
                            nc.sync.dma_start(
                                V_w[:, :, bw, :],
                                V[2 * w + bw].rearrange("(t p) n -> p t n", p=128),
                            )
                        VnT0 = vntp.tile([128, WB, DV], BF16, tag="vnt0")
                        VnT1 = vntp.tile([68, WB, DV], BF16, tag="vnt1")
                        nc.sync.dma_start(
                            VnT0[:],
                            VnT[2 * w : 2 * w + 2, 0:128, :].rearrange(
                                "b p d -> p b d"
                            ),
                        )
                        nc.sync.dma_start(
                            VnT1[:],
                            VnT[2 * w : 2 * w + 2, 128:WH, :].rearrange(
                                "b p d -> p b d"
                            ),
                        )
                        xatt_w = xap.tile([128, DAT, RW], F32R, tag="xatt")
                        for mt in range(DAT):
                            rows = _mt_rows(mt)
                            ps = psxv.tile([128, RW], F32, tag="psxv")
                            for kt in range(DVT):
                                nc.tensor.matmul(
                                    ps[:rows, :],
                                    WvT_sb[:, kt, mt * 128 : mt * 128 + rows],
                                    V_w[:, kt].rearrange("p b n -> p (b n)"),
                                    start=(kt == 0),
                                    stop=(kt == DVT - 1),
                                )
                            t1 = tmpp.tile([128, WB, WH], F32, tag="t1")
                            nc.scalar.activation(
                                t1[:rows].rearrange("p b n -> p (b n)"),
                                ps[:rows, :],
                                AF.Tanh, bias=bva_sb[:rows, mt : mt + 1],
                            )
                            nc.vector.tensor_tensor(
                                t1[:rows],
                                t1[:rows],
                                xqT_sb[:rows, mt, 2 * w : 2 * w + 2, None]
                                .to_broadcast([rows, WB, WH]),
                                mybir.AluOpType.mult,
                            )
                            nc.scalar.activation(
                                xatt_w[:rows, mt, :],
                                t1[:rows].rearrange("p b n -> p (b n)"),
                                AF.Tanh,
                            )
                        # wgt = xatt @ Watt.T  -> psum [G, RW]
                        psw = pswg.tile([G, RW], F32, tag="pswg")
                        for mt in range(DAT):
                            rows = _mt_rows(mt)
                            nc.tensor.matmul(
                                psw[:, :],
                                WattT_sb[:rows, mt, :],
                                xatt_w[:rows, mt, :],
                                start=(mt == 0),
                                stop=(mt == DAT - 1),
                            )
                        # biased logits [g, b, n]; DMA out per window
                        wgt_w = tmpp.tile([G, WB, WH], F32, tag="wgtw", bufs=2)
                        nc.scalar.activation(
                            wgt_w[:],
                            psw[:].rearrange("g (b n) -> g b n", b=WB),
                            AF.Identity, bias=batt_sb[:],
                        )
                        nc.sync.dma_start(
                            WGT_O[2 * w : 2 * w + 2].rearrange("b g n -> g b n"),
                            wgt_w[:],
                        )
                        # softmax over n in [g, b, n] layout (all offsets 0)
                        mx = tmpp.tile([G, WB, 1], F32, tag="mx", bufs=2)
                        sm = tmpp.tile([G, WB, 1], F32, tag="sm", bufs=2)
                        rs = tmpp.tile([G, WB, 1], F32, tag="rs", bufs=2)
                        att_w = tmpp.tile([G, WB, WH], F32, tag="attw", bufs=2)
                        nc.vector.reduce_max(mx[:], wgt_w[:], axis=AX.X, negate=True)
                        nc.vector.tensor_tensor(
                            att_w[:], wgt_w[:], mx[:].to_broadcast([G, WB, WH]),
                            mybir.AluOpType.add,
                        )
                        nc.scalar.activation(att_w[:], att_w[:], AF.Exp)
                        nc.vector.reduce_sum(sm[:], att_w[:], axis=AX.X)
                        nc.vector.reciprocal(rs[:], sm[:])
                        nc.vector.tensor_tensor(
                            att_w[:], att_w[:], rs[:].to_broadcast([G, WB, WH]),
                            mybir.AluOpType.mult,
                        )
                        # transpose att -> 4 block-column lhsT tiles [n, (bw, g)]
                        # (zero cols for the other batch; keeps PSUM dst at
                        # base partition 0, which the fp32r matmul requires)
                        attTs = []
                        for bw in range(WB):
                            a0 = tmpp.tile([128, 2 * WB], BF16,
                                           tag=f"attT0{bw}", bufs=2)
                            a1 = tmpp.tile([68, 2 * WB], BF16,
                                           tag=f"attT1{bw}", bufs=2)
                            nc.vector.memset(a0[:], 0.0)
                            nc.vector.memset(a1[:], 0.0)
                            pt0 = pstr.tile([128, G], F32, tag="pstr")
                            nc.tensor.transpose(
                                pt0[:], att_w[:, bw, 0:128], ident[0:G, 0:G]
                            )
                            nc.vector.tensor_copy(
                                a0[:, 2 * bw : 2 * bw + 2], pt0[:]
                            )
                            pt1 = pstr.tile([128, G], F32, tag="pstr")
                            nc.tensor.transpose(
                                pt1[0:68, :], att_w[:, bw, 128:WH], ident[0:G, 0:G]
                            )
                            nc.vector.tensor_copy(
                                a1[:, 2 * bw : 2 * bw + 2], pt1[0:68, :]
                            )
                            attTs.append((a0, a1))
                        # v_att: accumulate all (bw, n-chunk) into psum [4, 512]
                        vatt_w = tmpp.tile([2 * WB, DV], BF16, tag="vattw", bufs=1)
                        for ch in range(DV // 512):
                            pv = psva.tile([2 * WB, 512], F32, tag="psva")
                            for bw in range(WB):
                                a0, a1 = attTs[bw]
                                nc.tensor.matmul(
                                    pv[:, :], a0[:, :],
                                    VnT0[:, bw, ch * 512 : (ch + 1) * 512],
                                    start=(bw == 0), stop=False,
                                )
                                nc.tensor.matmul(
                                    pv[:, :], a1[:, :],
                                    VnT1[:, bw, ch * 512 : (ch + 1) * 512],
                                    start=False, stop=(bw == WB - 1),
                                )
                            nc.vector.tensor_copy(
                                vatt_w[:, ch * 512 : (ch + 1) * 512], pv[:]
                            )
                        nc.sync.dma_start(
                            vatt_loc[2 * w : 2 * w + 2].rearrange(
                                "b g d -> (b g) d"
                            ),
                            vatt_w[:],
                        )
                        # per-window AllGather (overlaps later windows)
                        ccw = nc.gpsimd.collective_compute(
                            "AllGather", mybir.AluOpType.bypass,
                            replica_groups=[list(range(NCORES))],
                            ins=[vatt_loc[2 * w : 2 * w + 2]],
                            outs=[vatt_all[w]],
                        )
                        ccv.append(ccw)

            # ================= phase C: fusion + classifier =================
            with (
                tc.tile_pool(name="ph2", bufs=1) as ph2,
                tc.tile_pool(name="ph2t", bufs=2) as ph2t,
                tc.tile_pool(name="psf", bufs=2, space="PSUM") as psf,
                tc.tile_pool(name="psx", bufs=2, space="PSUM") as psx,
            ):
                # v_att (both glimpses), batches in PERM order
                vatt_bg = ph2.tile([64, GVT, 128], BF16)
                d3 = nc.sync.dma_start(
                    vatt_bg[:],
                    vatt_all.rearrange("w c g (t p) -> (w c) (g t) p", p=128),
                )
                for ccw in ccv:
                    add_dep_helper(d3.ins, ccw.ins, reason="vatt allgather -> read")
                identb = ph2.tile([64, 64], BF16)
                nc.vector.tensor_copy(identb[:], ident[0:64, 0:64])
                for t in range(GVT):
                    ptv = pstr.tile([128, 64], BF16, tag="pstr")
                    nc.tensor.transpose(ptv[:], vatt_bg[:, t, :], identb[:])
                    nc.vector.tensor_copy(vattT_sb[:, t, :], ptv[:])

                # v_fus / z
                zT = ph2.tile([128, 3, B], BF16)
                nc.vector.memset(zT[:].bitcast(F32), 0.0)
                vf = ph2t.tile([128, 3, B], F32, tag="vf")
                for ch, rows in enumerate((128, 128, GSL - 256)):
                    psv = psf.tile([128, B], F32, tag="psv")
                    for kt in range(GVT):
                        nc.tensor.matmul(
                            psv[:rows, :],
                            WvfT_sb[:, kt, ch * 128 : ch * 128 + rows],
                            vattT_sb[:, kt, :],
                            start=(kt == 0), stop=(kt == GVT - 1),
                        )
                    nc.scalar.activation(
                        vf[:rows, ch, :], psv[:rows, :], AF.Tanh,
                        bias=bvf_sb[:rows, ch : ch + 1],
                    )
                    nc.vector.tensor_tensor(
                        zT[:rows, ch, :], vf[:rows, ch, :], qf[:rows, ch, :],
                        mybir.AluOpType.mult,
                    )
                # classifier partials: x [B, NCLS] (rows in PERM order)
                x_sb = ph2.tile([64, NCLS], F32)
                for ch in range(NCH):
                    px = psx.tile([64, NCW], F32, tag="px")
                    for kt in range(3):
                        nc.tensor.matmul(
                            px[:, :],
                            zT[:, kt, :],
                            WcT_sb[:, kt, ch * NCW : (ch + 1) * NCW],
                            start=(kt == 0), stop=(kt == 2),
                        )
                    nc.vector.tensor_copy(
                        x_sb[:, ch * NCW : (ch + 1) * NCW], px[:]
                    )
                nc.sync.dma_start(X_O[:], x_sb[:])

    nc.compile()
    return nc


_NC_CACHE = None


def _get_nc():
    global _NC_CACHE
    if _NC_CACHE is None:
        _NC_CACHE = build()
    return _NC_CACHE


def _prep_inputs(input_q, input_v, Wv_att, bv_att, Wq_att, bq_att, Watt, batt,
                 Wv_fus, bv_fus, Wq_fus, bq_fus, Wc, bc):
    """Host-side prep: transposes, pads, per-core slices."""
    f = np.float32
    bf = ml_dtypes.bfloat16
    input_q = np.ascontiguousarray(input_q, dtype=f)
    input_v = np.ascontiguousarray(input_v, dtype=f)

    v_flat = input_v.reshape(B, DV, WH)
    WvT_h = np.ascontiguousarray(Wv_att.T, dtype=bf)             # [2048, 1200]
    bva_h = np.zeros(1280, f)
    bva_h[:DA] = bv_att
    qT_h = np.zeros((DQP, B), np.float32)
    qT_h[:DQ] = input_q.T
    qT2_h = np.ascontiguousarray(qT_h[:, PERM]).astype(bf)
    WqaT_h = np.zeros((DQP, DA), bf)
    WqaT_h[:DQ] = Wq_att.T
    bqa_h = np.zeros(1280, f)
    bqa_h[:DA] = bq_att
    WattT_h = np.zeros((1280, G), f)
    WattT_h[:DA] = Watt.T
    batt_h = np.ascontiguousarray(batt.reshape(G, 1), dtype=f)

    in_maps = []
    for c in range(NCORES):
        bs = c * BL
        vloc = np.ascontiguousarray(v_flat[bs : bs + BL], dtype=bf)
        vnt = np.ascontiguousarray(
            v_flat[bs : bs + BL].transpose(0, 2, 1), dtype=bf)

        qTl_h = np.zeros((DQP, BL), bf)
        qTl_h[:DQ] = input_q[bs : bs + BL].T

        s0 = c * GSL
        g = s0 // DH
        h0 = s0 - g * DH
        WvfT_h = np.zeros((2 * DV, GSL), bf)
        WvfT_h[g * DV : (g + 1) * DV] = Wv_fus[g, h0 : h0 + GSL].T
        bvf_h = np.zeros(384, f)
        bvf_h[:GSL] = bv_fus[g, h0 : h0 + GSL]
        WqfT_h = np.zeros((DQP, GSL), bf)
        WqfT_h[:DQ] = Wq_fus[s0 : s0 + GSL].T
        bqf_h = np.zeros(384, f)
        bqf_h[:GSL] = bq_fus[s0 : s0 + GSL]
        WcT_h = np.zeros((384, NCLS), bf)
        WcT_h[:GSL] = Wc[:, s0 : s0 + GSL].T

        in_maps.append({
            "v": vloc, "vnt": vnt, "wvt": WvT_h, "bva": bva_h,
            "wqat": WqaT_h, "bqa": bqa_h, "qtl": qTl_h, "qt2": qT2_h,
            "watt": WattT_h, "batt": batt_h,
            "wvft": WvfT_h, "bvf": bvf_h, "wqft": WqfT_h, "bqf": bqf_h,
            "wct": WcT_h,
        })
    return in_maps


def kernel(**inputs):
    nc = _get_nc()
    in_maps = _prep_inputs(**{k: np.asarray(v) for k, v in inputs.items()})
    res = run_bass_kernel_spmd(nc, in_maps, core_ids=list(range(NCORES)))
    x = np.zeros((B, NCLS), np.float32)
    wgt = np.zeros((B, G, WH), np.float32)
    for c in range(NCORES):
        r = res.results[c]
        x += r["x_part"]
        wgt[c * BL : (c + 1) * BL] = r["wgt_out"]
    # un-permute: x_part rows are in PERM order (row j -> batch PERM[j])
    xo = np.zeros_like(x)
    xo[PERM] = x
    xo += np.asarray(inputs["bc"], np.float32)[None, :]
    return xo, wgt


# revision 49
# speedup vs baseline: 1.0982x; 1.0531x over previous
"""Trainium2 Bass kernel for nn_AbstractAtt (MLB-style 2-glimpse attention).

Strategy (8 NeuronCores, one SPMD NEFF):
- Phase 1 (attention) data-parallel over batch: 8 batches/core.
  xv = tanh(conv1x1(input_v)) computed transposed (DA on partitions) with
  bf16 matmuls (fp32 accumulate); xatt = tanh(xv * xq); wgt = xatt @ Watt.T
  (+batt) in fp32r; softmax over spatial; v_att = att-weighted sum of
  input_v in fp32r. xq is tensor-parallel over DA (150 rows/core) + AllToAll.
- Per-window AllGather of v_att (4 small gathers overlap compute).
- Phase 2 tensor-parallel over the 2400-dim fusion axis (300 cols/core),
  batches processed in gathered (window, core, b) order; host un-permutes.
  v_fus/q_fus slices, z = v_fus*q_fus, partial logits z @ Wc.T slice.
  Host sums the 8 partial logit tensors and adds bc.
"""
import numpy as np
import ml_dtypes

import concourse.bacc as bacc
import concourse.mybir as mybir
import concourse.tile as tile
from concourse.tile import add_dep_helper
from concourse.masks import make_identity
from concourse.bass_utils import run_bass_kernel_spmd

F32 = mybir.dt.float32
F32R = mybir.dt.float32r
BF16 = mybir.dt.bfloat16
AF = mybir.ActivationFunctionType
AX = mybir.AxisListType

NCORES = 8
B, DV, WH = 64, 2048, 196
DQ, DA, G, DH, NCLS = 2400, 1200, 2, 1200, 3000
BL = B // NCORES          # 8 local batches
NW = 4                    # windows per core
WB = BL // NW             # 2 batches per window
RW = WB * WH              # 392 rows per window
DVT = DV // 128           # 16 k-tiles over channels
DAT = 10                  # DA m-tiles: 9*128 + 48
DA_LAST = DA - 9 * 128    # 48
DQP = 2432                # DQ padded to 19*128
DQT = DQP // 128          # 19
DSL = DA // NCORES        # 150 (xq tensor-parallel slice)
GSL = (G * DH) // NCORES  # 300 (phase-2 slice)
GVT = 2 * DVT             # 32 k-tiles over (g, d) for v_fus
NCH = 6                   # classifier N chunks
NCW = NCLS // NCH         # 500

# phase-2 batch order: j = 16w + 2c + bw  <->  b_global = 8c + 2w + bw
PERM = np.zeros(B, np.int64)
for _j in range(B):
    _w, _r = divmod(_j, 2 * NCORES)
    _c, _bw = divmod(_r, WB)
    PERM[_j] = BL * _c + WB * _w + _bw


def _mt_rows(mt):
    return 128 if mt < DAT - 1 else DA_LAST


def build():
    nc = bacc.Bacc(None, num_devices=NCORES)

    # ---- per-core external inputs ----
    V = nc.declare_dram_parameter("v", [BL, DV, WH], BF16, isOutput=False)
    VnT = nc.declare_dram_parameter("vnt", [BL, WH, DV], BF16, isOutput=False)
    WvT = nc.declare_dram_parameter("wvt", [DV, DA], BF16, isOutput=False)
    BVA = nc.declare_dram_parameter("bva", [1280], F32, isOutput=False)
    WqaT = nc.declare_dram_parameter("wqat", [DQP, DA], BF16, isOutput=False)
    BQA = nc.declare_dram_parameter("bqa", [1280], F32, isOutput=False)
    QTL = nc.declare_dram_parameter("qtl", [DQP, BL], BF16, isOutput=False)
    QT2 = nc.declare_dram_parameter("qt2", [DQP, B], BF16, isOutput=False)
    WattT = nc.declare_dram_parameter("watt", [1280, G], F32R, isOutput=False)
    BATT = nc.declare_dram_parameter("batt", [G, 1], F32, isOutput=False)
    WvfT = nc.declare_dram_parameter("wvft", [2 * DV, GSL], BF16, isOutput=False)
    BVF = nc.declare_dram_parameter("bvf", [384], F32, isOutput=False)
    WqfT = nc.declare_dram_parameter("wqft", [DQP, GSL], BF16, isOutput=False)
    BQF = nc.declare_dram_parameter("bqf", [384], F32, isOutput=False)
    WcT = nc.declare_dram_parameter("wct", [384, NCLS], BF16, isOutput=False)

    # ---- per-core external outputs (x_part rows in PERM order) ----
    WGT_O = nc.declare_dram_parameter("wgt_out", [BL, G, WH], F32, isOutput=True)
    X_O = nc.declare_dram_parameter("x_part", [B, NCLS], F32, isOutput=True)

    # ---- internal DRAM for collectives ----
    vatt_loc = nc.dram_tensor("vatt_loc", [BL, G, DV], BF16)
    # gathered per window: [w][(core, bw)][g][d]
    vatt_all = nc.dram_tensor(
        "vatt_all", [NW, 2 * NCORES, G, DV], BF16, addr_space="Shared"
    )

    with tile.TileContext(nc) as tc:
        with (
            tc.tile_pool(name="const", bufs=1) as const,
            tc.tile_pool(name="ph2w", bufs=1) as ph2w,
            tc.tile_pool(name="pstr", bufs=2, space="PSUM") as pstr,
        ):
            ident = const.tile([128, 128], F32)
            make_identity(nc, ident)
            # persistent small tiles
            xqT_sb = const.tile([128, DAT, BL], F32)      # xq for local batches
            WattT_sb = const.tile([128, DAT, G], F32R)
            bva_sb = const.tile([128, DAT], F32)
            batt_sb = const.tile([G, 1], F32)
            qf = const.tile([128, 3, B], F32)             # q_fus (PERM order)
            # phase-2 prefetch tiles (DMA'd on the scalar queue during phase A)
            WqfT_sb = ph2w.tile([128, DQT, GSL], BF16)
            qT2_sb = ph2w.tile([128, DQT, B], BF16)
            bqf_sb = ph2w.tile([128, 3], F32)
            WvfT_sb = ph2w.tile([128, GVT, GSL], BF16)
            WcT_sb = ph2w.tile([128, 3, NCLS], BF16)
            bvf_sb = ph2w.tile([128, 3], F32)
            vattT_sb = ph2w.tile([128, GVT, B], BF16)

            nc.sync.dma_start(
                WattT_sb[:], WattT.rearrange("(t p) g -> p t g", p=128)
            )
            nc.sync.dma_start(bva_sb[:], BVA.rearrange("(t p) -> p t", p=128))
            nc.sync.dma_start(batt_sb[:], BATT[:])

            # ===== phase A: xq (tensor-parallel over DA) + early q_fus ======
            with (
                tc.tile_pool(name="xqp", bufs=1) as xqp,
                tc.tile_pool(name="psxq", bufs=2, space="PSUM") as psxq,
            ):
                WqaT_sb = xqp.tile([128, DQT, DA], BF16)
                qTl_sb = xqp.tile([128, DQT, BL], BF16)
                nc.scalar.dma_start(
                    qTl_sb[:], QTL.rearrange("(t p) b -> p t b", p=128)
                )
                bqa_sb = xqp.tile([128, DAT], F32)
                nc.scalar.dma_start(bqa_sb[:], BQA.rearrange("(c p) -> p c", p=128))
                # per-kt WqaT loads so the first xq matmul starts after ~0.3MB
                for kt in range(DQT):
                    nc.scalar.dma_start(
                        WqaT_sb[:, kt, :], WqaT[kt * 128 : (kt + 1) * 128, :]
                    )
                for mt in range(DAT):
                    rows = _mt_rows(mt)
                    ps = psxq.tile([128, BL], F32, tag="psxq")
                    for kt in range(DQT):
                        nc.tensor.matmul(
                            ps[:rows, :],
                            WqaT_sb[:, kt, mt * 128 : mt * 128 + rows],
                            qTl_sb[:, kt, :],
                            start=(kt == 0),
                            stop=(kt == DQT - 1),
                        )
                    nc.scalar.activation(
                        xqT_sb[:rows, mt, :], ps[:rows, :], AF.Tanh,
                        bias=bqa_sb[:rows, mt : mt + 1],
                    )
                # prefetch phase-2 weights on the scalar queue (behind the
                # xq inputs, ahead of nothing critical)
                nc.scalar.dma_start(
                    WqfT_sb[:], WqfT.rearrange("(t p) m -> p t m", p=128)
                )
                nc.scalar.dma_start(
                    qT2_sb[:], QT2.rearrange("(t p) b -> p t b", p=128)
                )
                nc.scalar.dma_start(bqf_sb[:], BQF.rearrange("(c p) -> p c", p=128))
                nc.scalar.dma_start(
                    WvfT_sb[:], WvfT.rearrange("(t p) m -> p t m", p=128)
                )
                nc.scalar.dma_start(
                    WcT_sb[:], WcT.rearrange("(t p) n -> p t n", p=128)
                )
                nc.scalar.dma_start(bvf_sb[:], BVF.rearrange("(c p) -> p c", p=128))
                # early q_fus (PERM batch order via QT2); weights prefetched
                # on the scalar DMA queue so the sync queue streams V/WvT.
                for ch, rows in enumerate((128, 128, GSL - 256)):
                    psq = psxq.tile([128, B], F32, tag="psxq")
                    for kt in range(DQT):
                        nc.tensor.matmul(
                            psq[:rows, :],
                            WqfT_sb[:, kt, ch * 128 : ch * 128 + rows],
                            qT2_sb[:, kt, :],
                            start=(kt == 0), stop=(kt == DQT - 1),
                        )
                    nc.scalar.activation(
                        qf[:rows, ch, :], psq[:rows, :], AF.Tanh,
                        bias=bqf_sb[:rows, ch : ch + 1],
                    )

            # ===== phase B: windows (xv -> xatt -> wgt -> att -> v_att) =====
            ccv = []
            with tc.tile_pool(name="wvtp", bufs=1) as wvtp:
                WvT_sb = wvtp.tile([128, DVT, DA], BF16)
                for kt in range(DVT):
                    nc.sync.dma_start(
                        WvT_sb[:, kt, :], WvT[kt * 128 : (kt + 1) * 128, :]
                    )
                with (
                    tc.tile_pool(name="vwin", bufs=2) as vwp,
                    tc.tile_pool(name="vntp", bufs=1) as vntp,
                    tc.tile_pool(name="xatp", bufs=1) as xap,
                    tc.tile_pool(name="tmpp", bufs=3) as tmpp,
                    tc.tile_pool(name="psxv", bufs=3, space="PSUM") as psxv,
                    tc.tile_pool(name="pswg", bufs=1, space="PSUM") as pswg,
                    tc.tile_pool(name="psva", bufs=2, space="PSUM") as psva,
                ):
                    for w in range(NW):
                        V_w = vwp.tile([128, DVT, WB, WH], BF16, tag="vw")
                        for bw in range(WB):
                            nc.sync.dma_start(
                                V_w[:, :, bw, :],
                                V[2 * w + bw].rearrange("(t p) n -> p t n", p=128),
                            )
                        VnT0 = vntp.tile([128, WB, DV], BF16, tag="vnt0")
                        VnT1 = vntp.tile([68, WB, DV], BF16, tag="vnt1")
                        nc.sync.dma_start(
                            VnT0[:],
                            VnT[2 * w : 2 * w + 2, 0:128, :].rearrange(
                                "b p d -> p b d"
                            ),
                        )
                        nc.sync.dma_start(
                            VnT1[:],
                            VnT[2 * w : 2 * w + 2, 128:WH, :].rearrange(
                                "b p d -> p b d"
                            ),
                        )
                        xatt_w = xap.tile([128, DAT, RW], F32R, tag="xatt")
                        for mt in range(DAT):
                            rows = _mt_rows(mt)
                            ps = psxv.tile([128, RW], F32, tag="psxv")
                            for kt in range(DVT):
                                nc.tensor.matmul(
                                    ps[:rows, :],
                                    WvT_sb[:, kt, mt * 128 : mt * 128 + rows],
                                    V_w[:, kt].rearrange("p b n -> p (b n)"),
                                    start=(kt == 0),
                                    stop=(kt == DVT - 1),
                                )
                            t1 = tmpp.tile([128, WB, WH], F32, tag="t1")
                            nc.scalar.activation(
                                t1[:rows].rearrange("p b n -> p (b n)"),
                                ps[:rows, :],
                                AF.Tanh, bias=bva_sb[:rows, mt : mt + 1],
                            )
                            nc.vector.tensor_tensor(
                                t1[:rows],
                                t1[:rows],
                                xqT_sb[:rows, mt, 2 * w : 2 * w + 2, None]
                                .to_broadcast([rows, WB, WH]),
                                mybir.AluOpType.mult,
                            )
                            nc.scalar.activation(
                                xatt_w[:rows, mt, :],
                                t1[:rows].rearrange("p b n -> p (b n)"),
                                AF.Tanh,
                            )
                        # wgt = xatt @ Watt.T  -> psum [G, RW]
                        psw = pswg.tile([G, RW], F32, tag="pswg")
                        for mt in range(DAT):
                            rows = _mt_rows(mt)
                            nc.tensor.matmul(
                                psw[:, :],
                                WattT_sb[:rows, mt, :],
                                xatt_w[:rows, mt, :],
                                start=(mt == 0),
                                stop=(mt == DAT - 1),
                            )
                        # biased logits [g, b, n]; DMA out per window
                        wgt_w = tmpp.tile([G, WB, WH], F32, tag="wgtw", bufs=2)
                        nc.scalar.activation(
                            wgt_w[:],
                            psw[:].rearrange("g (b n) -> g b n", b=WB),
                            AF.Identity, bias=batt_sb[:],
                        )
                        nc.sync.dma_start(
                            WGT_O[2 * w : 2 * w + 2].rearrange("b g n -> g b n"),
                            wgt_w[:],
                        )
                        # softmax over n in [g, b, n] layout (all offsets 0)
                        mx = tmpp.tile([G, WB, 1], F32, tag="mx", bufs=2)
                        sm = tmpp.tile([G, WB, 1], F32, tag="sm", bufs=2)
                        rs = tmpp.tile([G, WB, 1], F32, tag="rs", bufs=2)
                        att_w = tmpp.tile([G, WB, WH], F32, tag="attw", bufs=2)
                        nc.vector.reduce_max(mx[:], wgt_w[:], axis=AX.X, negate=True)
                        nc.vector.tensor_tensor(
                            att_w[:], wgt_w[:], mx[:].to_broadcast([G, WB, WH]),
                            mybir.AluOpType.add,
                        )
                        nc.scalar.activation(att_w[:], att_w[:], AF.Exp)
                        nc.vector.reduce_sum(sm[:], att_w[:], axis=AX.X)
                        nc.vector.reciprocal(rs[:], sm[:])
                        nc.vector.tensor_tensor(
                            att_w[:], att_w[:], rs[:].to_broadcast([G, WB, WH]),
                            mybir.AluOpType.mult,
                        )
                        # transpose att -> 4 block-column lhsT tiles [n, (bw, g)]
                        # (zero cols for the other batch; keeps PSUM dst at
                        # base partition 0, which the fp32r matmul requires)
                        attTs = []
                        for bw in range(WB):
                            a0 = tmpp.tile([128, 2 * WB], BF16,
                                           tag=f"attT0{bw}", bufs=2)
                            a1 = tmpp.tile([68, 2 * WB], BF16,
                                           tag=f"attT1{bw}", bufs=2)
                            nc.vector.memset(a0[:], 0.0)
                            nc.vector.memset(a1[:], 0.0)
                            pt0 = pstr.tile([128, G], F32, tag="pstr")
                            nc.tensor.transpose(
                                pt0[:], att_w[:, bw, 0:128], ident[0:G, 0:G]
                            )
                            nc.vector.tensor_copy(
                                a0[:, 2 * bw : 2 * bw + 2], pt0[:]
                            )
                            pt1 = pstr.tile([128, G], F32, tag="pstr")
                            nc.tensor.transpose(
                                pt1[0:68, :], att_w[:, bw, 128:WH], ident[0:G, 0:G]
                            )
                            nc.vector.tensor_copy(
                                a1[:, 2 * bw : 2 * bw + 2], pt1[0:68, :]
                            )
                            attTs.append((a0, a1))
                        # v_att: accumulate all (bw, n-chunk) into psum [4, 512]
                        vatt_w = tmpp.tile([2 * WB, DV], BF16, tag="vattw", bufs=1)
                        for ch in range(DV // 512):
                            pv = psva.tile([2 * WB, 512], F32, tag="psva")
                            for bw in range(WB):
                                a0, a1 = attTs[bw]
                                nc.tensor.matmul(
                                    pv[:, :], a0[:, :],
                                    VnT0[:, bw, ch * 512 : (ch + 1) * 512],
                                    start=(bw == 0), stop=False,
                                )
                                nc.tensor.matmul(
                                    pv[:, :], a1[:, :],
                                    VnT1[:, bw, ch * 512 : (ch + 1) * 512],
                                    start=False, stop=(bw == WB - 1),
                                )
                            nc.vector.tensor_copy(
                                vatt_w[:, ch * 512 : (ch + 1) * 512], pv[:]
                            )
                        nc.sync.dma_start(
                            vatt_loc[2 * w : 2 * w + 2].rearrange(
                                "b g d -> (b g) d"
                            ),
                            vatt_w[:],
                        )
                        # per-window AllGather (overlaps later windows)
                        ccw = nc.gpsimd.collective_compute(
                            "AllGather", mybir.AluOpType.bypass,
                            replica_groups=[list(range(NCORES))],
                            ins=[vatt_loc[2 * w : 2 * w + 2]],
                            outs=[vatt_all[w]],
                        )
                        ccv.append(ccw)

            # ================= phase C: fusion + classifier =================
            with (
                tc.tile_pool(name="ph2", bufs=1) as ph2,
                tc.tile_pool(name="ph2t", bufs=2) as ph2t,
                tc.tile_pool(name="psf", bufs=2, space="PSUM") as psf,
                tc.tile_pool(name="psx", bufs=2, space="PSUM") as psx,
            ):
                # v_att (both glimpses), batches in PERM order
                vatt_bg = ph2.tile([64, GVT, 128], BF16)
                d3 = nc.sync.dma_start(
                    vatt_bg[:],
                    vatt_all.rearrange("w c g (t p) -> (w c) (g t) p", p=128),
                )
                for ccw in ccv:
                    add_dep_helper(d3.ins, ccw.ins, reason="vatt allgather -> read")
                identb = ph2.tile([64, 64], BF16)
                nc.vector.tensor_copy(identb[:], ident[0:64, 0:64])
                for t in range(GVT):
                    ptv = pstr.tile([128, 64], BF16, tag="pstr")
                    nc.tensor.transpose(ptv[:], vatt_bg[:, t, :], identb[:])
                    nc.vector.tensor_copy(vattT_sb[:, t, :], ptv[:])

                # v_fus / z
                zT = ph2.tile([128, 3, B], BF16)
                nc.vector.memset(zT[:].bitcast(F32), 0.0)
                vf = ph2t.tile([128, 3, B], F32, tag="vf")
                for ch, rows in enumerate((128, 128, GSL - 256)):
                    psv = psf.tile([128, B], F32, tag="psv")
                    for kt in range(GVT):
                        nc.tensor.matmul(
                            psv[:rows, :],
                            WvfT_sb[:, kt, ch * 128 : ch * 128 + rows],
                            vattT_sb[:, kt, :],
                            start=(kt == 0), stop=(kt == GVT - 1),
                        )
                    nc.scalar.activation(
                        vf[:rows, ch, :], psv[:rows, :], AF.Tanh,
                        bias=bvf_sb[:rows, ch : ch + 1],
                    )
                    nc.vector.tensor_tensor(
                        zT[:rows, ch, :], vf[:rows, ch, :], qf[:rows, ch, :],
                        mybir.AluOpType.mult,
                    )
                # classifier partials: x [B, NCLS] (rows in PERM order)
                x_sb = ph2.tile([64, NCLS], F32)
                for ch in range(NCH):
                    px = psx.tile([64, NCW], F32, tag="px")
                    for kt in range(3):
                        nc.tensor.matmul(
                            px[:, :],
                            zT[:, kt, :],
                            WcT_sb[:, kt, ch * NCW : (ch + 1) * NCW],
                            start=(kt == 0), stop=(kt == 2),
                        )
                    nc.vector.tensor_copy(
                        x_sb[:, ch * NCW : (ch + 1) * NCW], px[:]
                    )
                nc.sync.dma_start(X_O[:], x_sb[:])

    nc.compile()
    return nc


_NC_CACHE = None


def _get_nc():
    global _NC_CACHE
    if _NC_CACHE is None:
        _NC_CACHE = build()
    return _NC_CACHE


def _prep_inputs(input_q, input_v, Wv_att, bv_att, Wq_att, bq_att, Watt, batt,
                 Wv_fus, bv_fus, Wq_fus, bq_fus, Wc, bc):
    """Host-side prep: transposes, pads, per-core slices."""
    f = np.float32
    bf = ml_dtypes.bfloat16
    input_q = np.ascontiguousarray(input_q, dtype=f)
    input_v = np.ascontiguousarray(input_v, dtype=f)

    v_flat = input_v.reshape(B, DV, WH)
    WvT_h = np.ascontiguousarray(Wv_att.T, dtype=bf)             # [2048, 1200]
    bva_h = np.zeros(1280, f)
    bva_h[:DA] = bv_att
    qT_h = np.zeros((DQP, B), np.float32)
    qT_h[:DQ] = input_q.T
    qT2_h = np.ascontiguousarray(qT_h[:, PERM]).astype(bf)
    WqaT_h = np.zeros((DQP, DA), bf)
    WqaT_h[:DQ] = Wq_att.T
    bqa_h = np.zeros(1280, f)
    bqa_h[:DA] = bq_att
    WattT_h = np.zeros((1280, G), f)
    WattT_h[:DA] = Watt.T
    batt_h = np.ascontiguousarray(batt.reshape(G, 1), dtype=f)

    in_maps = []
    for c in range(NCORES):
        bs = c * BL
        vloc = np.ascontiguousarray(v_flat[bs : bs + BL], dtype=bf)
        vnt = np.ascontiguousarray(
            v_flat[bs : bs + BL].transpose(0, 2, 1), dtype=bf)

        qTl_h = np.zeros((DQP, BL), bf)
        qTl_h[:DQ] = input_q[bs : bs + BL].T

        s0 = c * GSL
        g = s0 // DH
        h0 = s0 - g * DH
        WvfT_h = np.zeros((2 * DV, GSL), bf)
        WvfT_h[g * DV : (g + 1) * DV] = Wv_fus[g, h0 : h0 + GSL].T
        bvf_h = np.zeros(384, f)
        bvf_h[:GSL] = bv_fus[g, h0 : h0 + GSL]
        WqfT_h = np.zeros((DQP, GSL), bf)
        WqfT_h[:DQ] = Wq_fus[s0 : s0 + GSL].T
        bqf_h = np.zeros(384, f)
        bqf_h[:GSL] = bq_fus[s0 : s0 + GSL]
        WcT_h = np.zeros((384, NCLS), bf)
        WcT_h[:GSL] = Wc[:, s0 : s0 + GSL].T

        in_maps.append({
            "v": vloc, "vnt": vnt, "wvt": WvT_h, "bva": bva_h,
            "wqat": WqaT_h, "bqa": bqa_h, "qtl": qTl_h, "qt2": qT2_h,
            "watt": WattT_h, "batt": batt_h,
            "wvft": WvfT_h, "bvf": bvf_h, "wqft": WqfT_h, "bqf": bqf_h,
            "wct": WcT_h,
        })
    return in_maps


def kernel(**inputs):
    nc = _get_nc()
    in_maps = _prep_inputs(**{k: np.asarray(v) for k, v in inputs.items()})
    res = run_bass_kernel_spmd(nc, in_maps, core_ids=list(range(NCORES)))
    x = np.zeros((B, NCLS), np.float32)
    wgt = np.zeros((B, G, WH), np.float32)
    for c in range(NCORES):
        r = res.results[c]
        x += r["x_part"]
        wgt[c * BL : (c + 1) * BL] = r["wgt_out"]
    # un-permute: x_part rows are in PERM order (row j -> batch PERM[j])
    xo = np.zeros_like(x)
    xo[PERM] = x
    xo += np.asarray(inputs["bc"], np.float32)[None, :]
    return xo, wgt
